# revision 37
# baseline (speedup 1.0000x reference)
"""Trainium2 kernel for nn_DetectionLoss (YOLO-style detection loss).

Strategy (pure data parallel, batch sharded 8 ways):
  * The dominant cost is the focal loss over pred_scores [256,10,6300]
    (64.5 MB). target_scores is 0 everywhere except TOPK entries per batch
    row, so the focal sum splits into
        sum_all focal(x, t=0)  +  sum_special [focal(x,1) - focal(x,0)]
    The first term (16.1M elements) runs on the 8 NeuronCores; the second
    term touches only B*K = 1280 scalars and is folded in on the host.
  * focal(x, 0) = 0.25 * sigmoid(x)^2 * softplus(x). The shipped walrus
    activation tables have no softplus/sigmoid+exp combo in one table set,
    so everything is built from exp/ln (both live in
    natural_log_exp_and_others -> zero table switches):
        u  = exp(x)            (ACT)
        p  = ln(1 + u)         (ACT)    == softplus(x)
        d  = x - p             (DVE)    == log sigmoid(x)
        s2 = exp(2 d)          (ACT)    == sigmoid(x)^2
        acc += sum(0.25*s2*p)  (DVE scalar_tensor_tensor w/ accum)
  * The box loss uses only the TOPK=5 matched anchors per batch row
    (1280 of 1.6M anchor slots); it and the anchor top-k selection are
    O(B*A) index work on targets_bbox [256,4] and run on the host.
"""
import sys

import numpy as np

# ---------------------------------------------------------------- constants
_B, _C, _A = 256, 10, 6300
_NCORES = 8
_BLOC = _B // _NCORES            # 32 batch rows per core
_ROWS = 128                      # SBUF partitions
_FREE = _BLOC * _C * _A // _ROWS  # 15750 fp32 per partition
# Uneven chunking: small chunks at the ends (fast pipeline ramp in, short
# engine tail out), bigger in the middle; sums to 15750.
_CHUNKS = [1050, 1050, 2100, 2625, 2625, 2625, 2625, 1050]
_TOPK = 5
_LEVELS = [(8.0, 60, 80), (16.0, 30, 40), (32.0, 15, 20)]

_CACHE = {}


def _ensure_import_paths():
    try:
        import concourse  # noqa: F401
        return
    except ImportError:
        pass
    for p in ("/opt/trn_rl_repo", "/root/.axon_site/_ro/trn_rl_repo"):
        if p not in sys.path:
            sys.path.insert(0, p)
    import concourse  # noqa: F401


def _split_excess_waits(nc, max_waits=1):
    """Stock walrus CoreV3 codegen rejects instructions carrying more than
    one sem-wait ("Too many sync wait commands"). Move excess waits onto
    same-engine InstNoOps placed immediately before the offending
    instruction; engines execute in order so semantics are unchanged."""
    import bass_rust

    n_new = 0
    for f in nc.m.functions:
        for bb in f.blocks:
            insns = bb.instructions
            out = []
            changed = False
            for ins in insns:
                si = getattr(ins, "sync_info", None)
                waits = list(si.on_wait) if (si is not None and si.on_wait) else []
                if len(waits) > max_waits:
                    keep = waits[-max_waits:]
                    extra = waits[:-max_waits]
                    for j in range(0, len(extra), max_waits):
                        nop = bass_rust.InstNoOp(
                            name=f"{ins.name}_wsplit{n_new}", engine=ins.engine
                        )
                        nop.sync_info = bass_rust.SyncInfo(
                            on_wait=extra[j : j + max_waits], on_update=[]
                        )
                        out.append(nop)
                        n_new += 1
                    ins.sync_info = bass_rust.SyncInfo(
                        on_wait=keep, on_update=list(si.on_update)
                    )
                    changed = True
                out.append(ins)
            if changed:
                insns[:] = out
    return n_new


def _build_nc_raw():
    """Raw-Bass pipeline with hand-placed semaphores.

    Per chunk i (sizes _CHUNKS, all SBUF ring-buffered):
      SYNC: DMA x_i (contiguous)            -> dsem[i] += 16
      ACT : v = exp(-x_i)                      (in-order after dsem[i])
            q_i = ln(1+v)                   -> qsem += 1
            s2 = exp(-2 q_i)                -> ssem += 1   (ring WAR: wait tsem)
      DVE : p = x_i + q_i                   -> asem += 1   (wait qsem >= i+1)
            acc_i = sum(0.25*s2*p)          -> tsem += 1   (wait ssem >= i+1)
    then DVE tree-adds the accs -> fsem, SYNC DMAs the result out and clears
    every semaphore so the NEFF can be re-executed.

    vs the TileContext version this removes the multi-microsecond preamble
    (sem init + barriers), the ~15us teardown (drain + EVSEM butterfly +
    fragmented per-range semaphore resets) and per-instruction EVENT_SEM
    overhead. A dummy 1-element EXP at t=0 pulls the ACT table load off the
    critical path (it would otherwise serialize behind the first chunk DMA).
    """
    import concourse.bass as bass
    import concourse.mybir as mybir

    F32 = mybir.dt.float32
    AF = mybir.ActivationFunctionType
    OP = mybir.AluOpType

    nch = len(_CHUNKS)
    fmax = max(_CHUNKS)
    nc = bass.Bass()
    xs = [
        nc.dram_tensor(f"x{i}", [_ROWS, fsz], F32, kind="ExternalInput")
        for i, fsz in enumerate(_CHUNKS)
    ]
    acc_out = nc.dram_tensor("acc_out", [1, 1], F32, kind="ExternalOutput")

    import contextlib

    with contextlib.ExitStack() as ctx:
        def sb(name, cols=fmax):
            return ctx.enter_context(nc.sbuf_tensor(name, [_ROWS, cols], F32))

        nx, nq, ns, npp = 4, 5, 3, 2  # ring depths
        xt = [sb(f"sb_x{k}") for k in range(nx)]
        qt = [sb(f"sb_q{k}") for k in range(nq)]
        st = [sb(f"sb_s{k}") for k in range(ns)]
        pt = [sb(f"sb_p{k}") for k in range(npp)]
        vt = sb("sb_v")
        jt = sb("sb_j")
        at = sb("sb_a", 16)
        ps = ctx.enter_context(nc.psum_tensor("ps_tot", [1, len(_CHUNKS)], F32))
        dsem = [ctx.enter_context(nc.semaphore(f"d{i}")) for i in range(nch)]
        qsem = ctx.enter_context(nc.semaphore("qs"))
        ssem = ctx.enter_context(nc.semaphore("ss"))
        asem = ctx.enter_context(nc.semaphore("as"))
        tsem = ctx.enter_context(nc.semaphore("ts"))
        fsem = ctx.enter_context(nc.semaphore("fs"))
        msem = ctx.enter_context(nc.semaphore("ms"))
        csem = ctx.enter_context(nc.semaphore("cs"))
        osem = ctx.enter_context(nc.semaphore("os"))
        block = ctx.enter_context(nc.Block(no_gpsimd_drain=True))

        @block.sync
        def _(sync):
            for i in range(nch):
                if i >= nx:  # x ring WAR: slot free once DVE's add i-nx done
                    sync.wait_ge(asem, i - nx + 1)
                sync.dma_start(xt[i % nx][:, : _CHUNKS[i]], xs[i][:]).then_inc(
                    dsem[i], 16
                )
            sync.wait_ge(csem, 1)
            sync.dma_start(acc_out[:], at[0:1, 14:15]).then_inc(osem, 16)
            # clear the dead sems while the 4-byte out-DMA completion
            # notification (~2us) is in flight; osem itself last
            for s in dsem:
                sync.sem_clear(s)
            for s in (qsem, ssem, asem, tsem, fsem, msem, csem):
                sync.sem_clear(s)
            sync.wait_ge(osem, 16)
            sync.sem_clear(osem)

        @block.scalar
        def _(scalar):
            # dummy 1-elem EXP: forces the natural_log_exp table load at t=0
            # (scale=0 -> exp(0*garbage)=1; input value never matters)
            scalar.activation(jt[0:1, 0:1], vt[0:1, 0:1], AF.Exp, scale=0.0)
            for i, fsz in enumerate(_CHUNKS):
                xv = xt[i % nx][:, :fsz]
                qv = qt[i % nq][:, :fsz]
                sv = st[i % ns][:, :fsz]
                scalar.wait_ge(dsem[i], 16)
                scalar.activation(vt[:, :fsz], xv, AF.Exp, scale=-1.0)
                if i >= nq:  # q ring WAR: slot free once DVE's add i-nq done
                    scalar.wait_ge(asem, i - nq + 1)
                scalar.activation(qv, vt[:, :fsz], AF.Ln, bias=1.0).then_inc(
                    qsem, 1
                )
                if i >= ns:  # s2 ring WAR: slot free once stt i-ns retired
                    scalar.wait_ge(tsem, i - ns + 1)
                scalar.activation(sv, qv, AF.Exp, scale=-2.0).then_inc(ssem, 1)

        @block.vector
        def _(vector):
            for i, fsz in enumerate(_CHUNKS):
                xv = xt[i % nx][:, :fsz]
                qv = qt[i % nq][:, :fsz]
                sv = st[i % ns][:, :fsz]
                pv = pt[i % npp][:, :fsz]
                vector.wait_ge(qsem, i + 1)
                vector.tensor_add(pv, xv, qv).then_inc(asem, 1)
                vector.wait_ge(ssem, i + 1)
                vector.scalar_tensor_tensor(
                    out=jt[:, :fsz], in0=sv, scalar=0.25, in1=pv,
                    op0=OP.mult, op1=OP.mult, accum_out=at[:, i : i + 1],
                ).then_inc(tsem, 1)
            # The drain makes the walrus-inserted DVE_READ_ACCUMULATOR spills
            # (which trail each stt) visible before PE reads the acc columns.
            vector.drain().then_inc(fsem, 1)
            # PE sums the [128, nch] accs over partitions into PSUM [1, nch];
            # one tiny free-dim reduce finishes the scalar.
            vector.wait_ge(msem, 1)
            vector.tensor_reduce(
                at[0:1, 14:15], ps[0:1, :], mybir.AxisListType.X, OP.add
            ).then_inc(csem, 1)

        @block.tensor
        def _(tensor):
            ones = nc.const_aps.aps[(F32, 1.0)]
            tensor.wait_ge(fsem, 1)
            nc.tensor.matmul(
                ps[:], ones, at[:, : len(_CHUNKS)], start=True, stop=True
            ).then_inc(msem, 1)

    # Drop the Block-exit all-engine EVSEM barrier (~7us of tail): every
    # cross-engine ordering this kernel needs already flows through its own
    # semaphores into the final DMA + sem_clears.
    for f in nc.m.functions:
        for bb in f.blocks:
            bb.instructions[:] = [
                ins for ins in bb.instructions
                if not ins.name.startswith("aeb_barrier_")
            ]

    # Hoist the wait-free input-DMA issues and the table-load dummy above
    # bass's init barrier: the first chunks stream in and the ACT tables load
    # while the engines are still in their preamble (~4us off the ramp).
    # Both touch only state no other engine reads before the barrier.
    ET = mybir.EngineType
    blocks = {bb.name: bb for f in nc.m.functions for bb in f.blocks}
    main = blocks["main"]

    def hoist(engine, want, before_type):
        src = next(
            bb for name, bb in blocks.items()
            if name != "main" and any(
                getattr(i, "engine", None) == engine for i in bb.instructions
            )
        )
        moved = []
        for ins in list(src.instructions):
            if len(moved) >= want:
                break
            if isinstance(ins, before_type) and not (
                ins.sync_info and ins.sync_info.on_wait
            ):
                moved.append(ins)
                src.instructions.remove(ins)
            else:
                break
        pos = next(
            k for k, ins in enumerate(main.instructions)
            if type(ins).__name__ == "InstDrain" and ins.engine == engine
        )
        main.instructions[pos:pos] = moved

    import bass_rust

    hoist(ET.SP, 4, bass_rust.InstDMACopy)
    hoist(ET.Activation, 1, bass_rust.InstActivation)
    return nc


def _build_nc():
    import concourse.bass as bass
    import concourse.mybir as mybir
    import concourse.tile as tile

    F32 = mybir.dt.float32
    AF = mybir.ActivationFunctionType
    OP = mybir.AluOpType

    nc = bass.Bass()
    # One DRAM tensor per chunk: each is a fully contiguous [128, F] block of
    # the flat per-core buffer (the focal sum is order-invariant), so every
    # DMA is a pure sequential stream instead of a 63KB-strided gather.
    xs = [
        nc.dram_tensor(f"x{i}", [_ROWS, fsz], F32, kind="ExternalInput")
        for i, fsz in enumerate(_CHUNKS)
    ]
    acc_out = nc.dram_tensor("acc_out", [_ROWS, 1], F32, kind="ExternalOutput")

    nch = len(_CHUNKS)
    offs = [sum(_CHUNKS[:i]) for i in range(nch)]
    with tile.TileContext(nc) as tc:
        with (
            tc.tile_pool(name="xp", bufs=3) as xp,
            tc.tile_pool(name="vp", bufs=2) as vp,
            tc.tile_pool(name="qp", bufs=3) as qp,
            tc.tile_pool(name="pp", bufs=2) as pp,
            tc.tile_pool(name="sp", bufs=2) as sp,
            tc.tile_pool(name="jp", bufs=1) as jp,
            tc.tile_pool(name="ap", bufs=nch + 4) as ap,
        ):
            accs = []
            for i, fsz in enumerate(_CHUNKS):
                xt = xp.tile([_ROWS, fsz], F32, tag="x")
                nc.sync.dma_start(xt[:], xs[i][:])
                # ACT-only transcendental chain (no DVE feedback):
                #   v = exp(-x); q = ln(1+v) = softplus(-x); s2 = exp(-2q)
                vt = vp.tile([_ROWS, fsz], F32, tag="v")
                nc.scalar.activation(vt[:], xt[:], AF.Exp, scale=-1.0)
                qt = qp.tile([_ROWS, fsz], F32, tag="q")
                nc.scalar.activation(qt[:], vt[:], AF.Ln, bias=1.0)
                s2t = sp.tile([_ROWS, fsz], F32, tag="s")
                nc.scalar.activation(s2t[:], qt[:], AF.Exp, scale=-2.0)
                # p = softplus(x) = x + q; alternate DVE/GpSimd to keep the
                # DVE tail short (DVE also runs the fused multiply-reduce).
                pt = pp.tile([_ROWS, fsz], F32, tag="p")
                eng = nc.gpsimd if i % 2 == 0 and i != nch - 1 else nc.vector
                eng.tensor_add(pt[:], xt[:], qt[:])
                jt = jp.tile([_ROWS, fsz], F32, tag="j")
                acc = ap.tile([_ROWS, 1], F32, tag=f"acc{i}")
                nc.vector.scalar_tensor_tensor(
                    out=jt[:], in0=s2t[:], scalar=0.25, in1=pt[:],
                    op0=OP.mult, op1=OP.mult, accum_out=acc[:],
                )
                accs.append(acc)
            while len(accs) > 1:
                nxt = []
                for j in range(0, len(accs) - 1, 2):
                    t = ap.tile([_ROWS, 1], F32, tag=f"sum{len(accs)}_{j}")
                    nc.vector.tensor_add(t[:], accs[j][:], accs[j + 1][:])
                    nxt.append(t)
                if len(accs) % 2:
                    nxt.append(accs[-1])
                accs = nxt
            nc.sync.dma_start(acc_out[:], accs[0][:])

    _split_excess_waits(nc, max_waits=1)
    return nc


def _get_nc():
    if "nc" not in _CACHE:
        _ensure_import_paths()
        _CACHE["nc"] = _build_nc_raw()
    return _CACHE["nc"]


def _run_device(in_maps, trace=False, tmpdir=None):
    _ensure_import_paths()
    from concourse.bass_utils import run_bass_kernel_spmd

    return run_bass_kernel_spmd(
        _get_nc(), in_maps, core_ids=list(range(_NCORES)), trace=trace,
        tmpdir=tmpdir,
    )


# ------------------------------------------------------------- host helpers
def _make_in_maps(pred_scores):
    in_maps = []
    for c in range(_NCORES):
        flat = pred_scores[c * _BLOC : (c + 1) * _BLOC].reshape(-1)
        m, off = {}, 0
        for i, fsz in enumerate(_CHUNKS):
            n = _ROWS * fsz
            m[f"x{i}"] = flat[off : off + n].reshape(_ROWS, fsz)
            off += n
        in_maps.append(m)
    return in_maps


def _make_anchors():
    pts, strs = [], []
    for stride, h, w in _LEVELS:
        sx = np.arange(w, dtype=np.float32) + 0.5
        sy = np.arange(h, dtype=np.float32) + 0.5
        gy, gx = np.meshgrid(sy, sx, indexing="ij")
        pts.append(np.stack([gx, gy], -1).reshape(-1, 2))
        strs.append(np.full((h * w, 1), stride, dtype=np.float32))
    return np.concatenate(pts), np.concatenate(strs)


def _cxcywh_to_xyxy(b):
    cx, cy, w, h = b[..., 0], b[..., 1], b[..., 2], b[..., 3]
    return np.stack([cx - w / 2, cy - h / 2, cx + w / 2, cy + h / 2], axis=-1)


def _giou_elementwise(a, b):
    lt = np.maximum(a[..., :2], b[..., :2])
    rb = np.minimum(a[..., 2:], b[..., 2:])
    wh = np.maximum(rb - lt, 0.0)
    inter = wh[..., 0] * wh[..., 1]
    area_a = (a[..., 2] - a[..., 0]) * (a[..., 3] - a[..., 1])
    area_b = (b[..., 2] - b[..., 0]) * (b[..., 3] - b[..., 1])
    union = area_a + area_b - inter
    iou = inter / union
    lt_c = np.minimum(a[..., :2], b[..., :2])
    rb_c = np.maximum(a[..., 2:], b[..., 2:])
    wh_c = np.maximum(rb_c - lt_c, 0.0)
    area_c = wh_c[..., 0] * wh_c[..., 1]
    return iou - (area_c - union) / area_c


def _focal_f32(x, t):
    """Reference focal loss term, elementwise, f64 math on f32 inputs."""
    x = x.astype(np.float64)
    bce = np.maximum(x, 0.0) - x * t + np.log1p(np.exp(-np.abs(x)))
    pt = np.exp(-bce)
    return 0.25 * (1.0 - pt) ** 2 * bce


# ------------------------------------------------------------------- kernel
def kernel(pred_boxes, pred_scores, targets_bbox, targets_cls):
    pred_boxes = np.asarray(pred_boxes, dtype=np.float32)
    pred_scores = np.ascontiguousarray(np.asarray(pred_scores, dtype=np.float32))
    targets_bbox = np.asarray(targets_bbox, dtype=np.float32)
    targets_cls = np.asarray(targets_cls)

    # ---- device: sum of focal(x, t=0) over all of pred_scores ----
    res = _run_device(_make_in_maps(pred_scores))
    focal0_total = float(
        sum(float(r["acc_out"][0, 0]) for r in res.results)
    )

    # ---- host: top-k anchor matching (depends only on targets_bbox) ----
    anchors, stride_t = _make_anchors()                    # [A,2], [A,1] f32
    centers = anchors * stride_t                           # [A,2] f32
    diff = centers[None, :, :] - targets_bbox[:, None, :2]  # [B,A,2] f32
    dist = np.sqrt(diff[..., 0] * diff[..., 0] + diff[..., 1] * diff[..., 1])
    topk_idx = np.argpartition(dist, _TOPK, axis=1)[:, :_TOPK]  # [B,K]

    bi = np.arange(_B)[:, None]
    # ---- host: GIoU box loss on the K matched anchors per batch row ----
    pb_g = pred_boxes.transpose(0, 2, 1)[bi, topk_idx]      # [B,K,4] f32
    anc_g = anchors[topk_idx]                               # [B,K,2]
    str_g = stride_t[topk_idx]                              # [B,K,1]
    pred_cxcy = (anc_g + pb_g[..., :2]) * str_g
    pred_wh = np.exp(np.minimum(pb_g[..., 2:], 10.0)) * str_g
    decoded = np.concatenate([pred_cxcy, pred_wh], axis=-1).astype(np.float32)
    pred_xyxy = _cxcywh_to_xyxy(decoded)
    gt_xyxy = _cxcywh_to_xyxy(targets_bbox)[:, None, :]
    giou = _giou_elementwise(
        pred_xyxy.astype(np.float64),
        np.broadcast_to(gt_xyxy, pred_xyxy.shape).astype(np.float64),
    )
    loss_box = (1.0 - giou).mean(axis=1).mean()

    # ---- host: focal correction at the K matched (anchor, class) slots ----
    cls_idx = targets_cls.astype(np.int64)[:, None]         # [B,1]
    xg = pred_scores[bi, cls_idx, topk_idx]                 # [B,K] f32
    corr = (_focal_f32(xg, 1.0) - _focal_f32(xg, 0.0)).sum()

    loss_cls = (focal0_total + corr) / _B
    total = 5.0 * loss_box + 1.0 * loss_cls
    return (
        np.float32(total),
        np.float32(loss_box),
        np.float32(loss_cls),
    )


# revision 43
# speedup vs baseline: 1.0473x; 1.0473x over previous
"""Trainium2 kernel for nn_DetectionLoss (YOLO-style detection loss).

Strategy (pure data parallel, batch sharded 8 ways):
  * The dominant cost is the focal loss over pred_scores [256,10,6300]
    (64.5 MB). target_scores is 0 everywhere except TOPK entries per batch
    row, so the focal sum splits into
        sum_all focal(x, t=0)  +  sum_special [focal(x,1) - focal(x,0)]
    The first term (16.1M elements) runs on the 8 NeuronCores; the second
    term touches only B*K = 1280 scalars and is folded in on the host.
  * focal(x, 0) = 0.25 * sigmoid(x)^2 * softplus(x). The shipped walrus
    activation tables have no softplus/sigmoid+exp combo in one table set,
    so everything is built from exp/ln (both live in
    natural_log_exp_and_others -> zero table switches):
        u  = exp(x)            (ACT)
        p  = ln(1 + u)         (ACT)    == softplus(x)
        d  = x - p             (DVE)    == log sigmoid(x)
        s2 = exp(2 d)          (ACT)    == sigmoid(x)^2
        acc += sum(0.25*s2*p)  (DVE scalar_tensor_tensor w/ accum)
  * The box loss uses only the TOPK=5 matched anchors per batch row
    (1280 of 1.6M anchor slots); it and the anchor top-k selection are
    O(B*A) index work on targets_bbox [256,4] and run on the host.
"""
import sys

import numpy as np

# ---------------------------------------------------------------- constants
_B, _C, _A = 256, 10, 6300
_NCORES = 8
_BLOC = _B // _NCORES            # 32 batch rows per core
_ROWS = 128                      # SBUF partitions
_FREE = _BLOC * _C * _A // _ROWS  # 15750 fp32 per partition
# Uneven chunking: small chunks at the ends (fast pipeline ramp in, short
# engine tail out), bigger in the middle; sums to 15750.
_CHUNKS = [1050, 1050, 2100, 2625, 2625, 2625, 2625, 1050]
_TOPK = 5
_LEVELS = [(8.0, 60, 80), (16.0, 30, 40), (32.0, 15, 20)]

_CACHE = {}


def _ensure_import_paths():
    try:
        import concourse  # noqa: F401
        return
    except ImportError:
        pass
    for p in ("/opt/trn_rl_repo", "/root/.axon_site/_ro/trn_rl_repo"):
        if p not in sys.path:
            sys.path.insert(0, p)
    import concourse  # noqa: F401


def _split_excess_waits(nc, max_waits=1):
    """Stock walrus CoreV3 codegen rejects instructions carrying more than
    one sem-wait ("Too many sync wait commands"). Move excess waits onto
    same-engine InstNoOps placed immediately before the offending
    instruction; engines execute in order so semantics are unchanged."""
    import bass_rust

    n_new = 0
    for f in nc.m.functions:
        for bb in f.blocks:
            insns = bb.instructions
            out = []
            changed = False
            for ins in insns:
                si = getattr(ins, "sync_info", None)
                waits = list(si.on_wait) if (si is not None and si.on_wait) else []
                if len(waits) > max_waits:
                    keep = waits[-max_waits:]
                    extra = waits[:-max_waits]
                    for j in range(0, len(extra), max_waits):
                        nop = bass_rust.InstNoOp(
                            name=f"{ins.name}_wsplit{n_new}", engine=ins.engine
                        )
                        nop.sync_info = bass_rust.SyncInfo(
                            on_wait=extra[j : j + max_waits], on_update=[]
                        )
                        out.append(nop)
                        n_new += 1
                    ins.sync_info = bass_rust.SyncInfo(
                        on_wait=keep, on_update=list(si.on_update)
                    )
                    changed = True
                out.append(ins)
            if changed:
                insns[:] = out
    return n_new


def _build_nc_raw():
    """Raw-Bass pipeline with hand-placed semaphores.

    Per chunk i (sizes _CHUNKS, all SBUF ring-buffered):
      SYNC: DMA x_i (contiguous)            -> dsem[i] += 16
      ACT : v = exp(-x_i)                      (in-order after dsem[i])
            q_i = ln(1+v)                   -> qsem += 1
            s2 = exp(-2 q_i)                -> ssem += 1   (ring WAR: wait tsem)
      DVE : p = x_i + q_i                   -> asem += 1   (wait qsem >= i+1)
            acc_i = sum(0.25*s2*p)          -> tsem += 1   (wait ssem >= i+1)
    then DVE tree-adds the accs -> fsem, SYNC DMAs the result out and clears
    every semaphore so the NEFF can be re-executed.

    vs the TileContext version this removes the multi-microsecond preamble
    (sem init + barriers), the ~15us teardown (drain + EVSEM butterfly +
    fragmented per-range semaphore resets) and per-instruction EVENT_SEM
    overhead. A dummy 1-element EXP at t=0 pulls the ACT table load off the
    critical path (it would otherwise serialize behind the first chunk DMA).
    """
    import concourse.bass as bass
    import concourse.mybir as mybir

    F32 = mybir.dt.float32
    AF = mybir.ActivationFunctionType
    OP = mybir.AluOpType

    nch = len(_CHUNKS)
    fmax = max(_CHUNKS)
    nc = bass.Bass()
    xs = [
        nc.dram_tensor(f"x{i}", [_ROWS, fsz], F32, kind="ExternalInput")
        for i, fsz in enumerate(_CHUNKS)
    ]
    acc_out = nc.dram_tensor("acc_out", [1, 1], F32, kind="ExternalOutput")

    import contextlib

    with contextlib.ExitStack() as ctx:
        def sb(name, cols=fmax):
            return ctx.enter_context(nc.sbuf_tensor(name, [_ROWS, cols], F32))

        nx, nq, ns, npp = 4, 5, 3, 2  # ring depths
        xt = [sb(f"sb_x{k}") for k in range(nx)]
        qt = [sb(f"sb_q{k}") for k in range(nq)]
        st = [sb(f"sb_s{k}") for k in range(ns)]
        pt = [sb(f"sb_p{k}") for k in range(npp)]
        vt = sb("sb_v")
        jt = sb("sb_j")
        at = sb("sb_a", 16)
        ps = ctx.enter_context(nc.psum_tensor("ps_tot", [1, len(_CHUNKS)], F32))
        dsem = [ctx.enter_context(nc.semaphore(f"d{i}")) for i in range(nch)]
        qsem = ctx.enter_context(nc.semaphore("qs"))
        ssem = ctx.enter_context(nc.semaphore("ss"))
        asem = ctx.enter_context(nc.semaphore("as"))
        tsem = ctx.enter_context(nc.semaphore("ts"))
        fsem = ctx.enter_context(nc.semaphore("fs"))
        msem = ctx.enter_context(nc.semaphore("ms"))
        csem = ctx.enter_context(nc.semaphore("cs"))
        osem = ctx.enter_context(nc.semaphore("os"))
        bsem = ctx.enter_context(nc.semaphore("bs"))
        bsem_id = bsem.num
        block = ctx.enter_context(nc.Block(no_gpsimd_drain=True))

        @block.sync
        def _(sync):
            for i in range(nch):
                if i >= nx:  # x ring WAR: slot free once DVE's add i-nx done
                    sync.wait_ge(asem, i - nx + 1)
                sync.dma_start(xt[i % nx][:, : _CHUNKS[i]], xs[i][:]).then_inc(
                    dsem[i], 16
                )
            sync.wait_ge(csem, 1)
            sync.dma_start(acc_out[:], at[0:1, 14:15]).then_inc(osem, 16)
            # clear the dead sems while the 4-byte out-DMA completion
            # notification (~2us) is in flight; osem itself last
            for s in dsem:
                sync.sem_clear(s)
            for s in (qsem, ssem, asem, tsem, fsem, msem, csem, bsem):
                sync.sem_clear(s)
            sync.wait_ge(osem, 16)
            sync.sem_clear(osem)

        @block.scalar
        def _(scalar):
            # dummy 1-elem EXP: forces the natural_log_exp table load at t=0
            # (scale=0 -> exp(0*garbage)=1; input value never matters).
            # Waits bsem: stands in for the stripped init barrier, ordering
            # the gpsimd const-memsets before any ACT const-bias read.
            scalar.wait_ge(bsem, 1)
            scalar.activation(jt[0:1, 0:1], vt[0:1, 0:1], AF.Exp, scale=0.0)
            for i, fsz in enumerate(_CHUNKS):
                xv = xt[i % nx][:, :fsz]
                qv = qt[i % nq][:, :fsz]
                sv = st[i % ns][:, :fsz]
                scalar.wait_ge(dsem[i], 16)
                scalar.activation(vt[:, :fsz], xv, AF.Exp, scale=-1.0)
                if i >= nq:  # q ring WAR: slot free once DVE's add i-nq done
                    scalar.wait_ge(asem, i - nq + 1)
                scalar.activation(qv, vt[:, :fsz], AF.Ln, bias=1.0).then_inc(
                    qsem, 1
                )
                if i >= ns:  # s2 ring WAR: slot free once stt i-ns retired
                    scalar.wait_ge(tsem, i - ns + 1)
                scalar.activation(sv, qv, AF.Exp, scale=-2.0).then_inc(ssem, 1)

        @block.vector
        def _(vector):
            for i, fsz in enumerate(_CHUNKS):
                xv = xt[i % nx][:, :fsz]
                qv = qt[i % nq][:, :fsz]
                sv = st[i % ns][:, :fsz]
                pv = pt[i % npp][:, :fsz]
                vector.wait_ge(qsem, i + 1)
                vector.tensor_add(pv, xv, qv).then_inc(asem, 1)
                vector.wait_ge(ssem, i + 1)
                vector.scalar_tensor_tensor(
                    out=jt[:, :fsz], in0=sv, scalar=0.25, in1=pv,
                    op0=OP.mult, op1=OP.mult, accum_out=at[:, i : i + 1],
                ).then_inc(tsem, 1)
            # The drain makes the walrus-inserted DVE_READ_ACCUMULATOR spills
            # (which trail each stt) visible before PE reads the acc columns.
            vector.drain().then_inc(fsem, 1)
            # PE sums the [128, nch] accs over partitions into PSUM [1, nch];
            # one tiny free-dim reduce finishes the scalar.
            vector.wait_ge(msem, 1)
            vector.tensor_reduce(
                at[0:1, 14:15], ps[0:1, :], mybir.AxisListType.X, OP.add
            ).then_inc(csem, 1)

        @block.tensor
        def _(tensor):
            ones = nc.const_aps.aps[(F32, 1.0)]
            tensor.wait_ge(fsem, 1)
            nc.tensor.matmul(
                ps[:], ones, at[:, : len(_CHUNKS)], start=True, stop=True
            ).then_inc(msem, 1)

    import bass_rust

    # Replace bass's init all-engine barrier with one semaphore edge: the
    # last gpsimd const-memset incs bsem, the first ACT instruction waits on
    # it (the only real pre-barrier dependency in this kernel). Then drop
    # BOTH all-engine EVSEM barriers (init + Block-exit) — every remaining
    # cross-engine ordering flows through this kernel's own semaphores.
    ET = mybir.EngineType
    for f in nc.m.functions:
        for bb in f.blocks:
            if bb.name == "main":
                memsets = [
                    i for i in bb.instructions
                    if type(i).__name__ == "InstMemset" and i.engine == ET.Pool
                ]
                last = memsets[-1]
                upd = bass_rust.SyncUpdate(
                    sync_type="semaphore", id=bsem_id, update_value=1,
                    update_mode="sem-inc", ant_name="bs",
                )
                old = last.sync_info
                last.sync_info = bass_rust.SyncInfo(
                    on_wait=list(old.on_wait) if old else [],
                    on_update=(list(old.on_update) if old else []) + [upd],
                )
            bb.instructions[:] = [
                ins for ins in bb.instructions
                if "barrier_" not in ins.name
            ]
    return nc


def _build_nc():
    import concourse.bass as bass
    import concourse.mybir as mybir
    import concourse.tile as tile

    F32 = mybir.dt.float32
    AF = mybir.ActivationFunctionType
    OP = mybir.AluOpType

    nc = bass.Bass()
    # One DRAM tensor per chunk: each is a fully contiguous [128, F] block of
    # the flat per-core buffer (the focal sum is order-invariant), so every
    # DMA is a pure sequential stream instead of a 63KB-strided gather.
    xs = [
        nc.dram_tensor(f"x{i}", [_ROWS, fsz], F32, kind="ExternalInput")
        for i, fsz in enumerate(_CHUNKS)
    ]
    acc_out = nc.dram_tensor("acc_out", [_ROWS, 1], F32, kind="ExternalOutput")

    nch = len(_CHUNKS)
    offs = [sum(_CHUNKS[:i]) for i in range(nch)]
    with tile.TileContext(nc) as tc:
        with (
            tc.tile_pool(name="xp", bufs=3) as xp,
            tc.tile_pool(name="vp", bufs=2) as vp,
            tc.tile_pool(name="qp", bufs=3) as qp,
            tc.tile_pool(name="pp", bufs=2) as pp,
            tc.tile_pool(name="sp", bufs=2) as sp,
            tc.tile_pool(name="jp", bufs=1) as jp,
            tc.tile_pool(name="ap", bufs=nch + 4) as ap,
        ):
            accs = []
            for i, fsz in enumerate(_CHUNKS):
                xt = xp.tile([_ROWS, fsz], F32, tag="x")
                nc.sync.dma_start(xt[:], xs[i][:])
                # ACT-only transcendental chain (no DVE feedback):
                #   v = exp(-x); q = ln(1+v) = softplus(-x); s2 = exp(-2q)
                vt = vp.tile([_ROWS, fsz], F32, tag="v")
                nc.scalar.activation(vt[:], xt[:], AF.Exp, scale=-1.0)
                qt = qp.tile([_ROWS, fsz], F32, tag="q")
                nc.scalar.activation(qt[:], vt[:], AF.Ln, bias=1.0)
                s2t = sp.tile([_ROWS, fsz], F32, tag="s")
                nc.scalar.activation(s2t[:], qt[:], AF.Exp, scale=-2.0)
                # p = softplus(x) = x + q; alternate DVE/GpSimd to keep the
                # DVE tail short (DVE also runs the fused multiply-reduce).
                pt = pp.tile([_ROWS, fsz], F32, tag="p")
                eng = nc.gpsimd if i % 2 == 0 and i != nch - 1 else nc.vector
                eng.tensor_add(pt[:], xt[:], qt[:])
                jt = jp.tile([_ROWS, fsz], F32, tag="j")
                acc = ap.tile([_ROWS, 1], F32, tag=f"acc{i}")
                nc.vector.scalar_tensor_tensor(
                    out=jt[:], in0=s2t[:], scalar=0.25, in1=pt[:],
                    op0=OP.mult, op1=OP.mult, accum_out=acc[:],
                )
                accs.append(acc)
            while len(accs) > 1:
                nxt = []
                for j in range(0, len(accs) - 1, 2):
                    t = ap.tile([_ROWS, 1], F32, tag=f"sum{len(accs)}_{j}")
                    nc.vector.tensor_add(t[:], accs[j][:], accs[j + 1][:])
                    nxt.append(t)
                if len(accs) % 2:
                    nxt.append(accs[-1])
                accs = nxt
            nc.sync.dma_start(acc_out[:], accs[0][:])

    _split_excess_waits(nc, max_waits=1)
    return nc


def _get_nc():
    if "nc" not in _CACHE:
        _ensure_import_paths()
        _CACHE["nc"] = _build_nc_raw()
    return _CACHE["nc"]


def _run_device(in_maps, trace=False, tmpdir=None):
    _ensure_import_paths()
    from concourse.bass_utils import run_bass_kernel_spmd

    return run_bass_kernel_spmd(
        _get_nc(), in_maps, core_ids=list(range(_NCORES)), trace=trace,
        tmpdir=tmpdir,
    )


# ------------------------------------------------------------- host helpers
def _make_in_maps(pred_scores):
    in_maps = []
    for c in range(_NCORES):
        flat = pred_scores[c * _BLOC : (c + 1) * _BLOC].reshape(-1)
        m, off = {}, 0
        for i, fsz in enumerate(_CHUNKS):
            n = _ROWS * fsz
            m[f"x{i}"] = flat[off : off + n].reshape(_ROWS, fsz)
            off += n
        in_maps.append(m)
    return in_maps


def _make_anchors():
    pts, strs = [], []
    for stride, h, w in _LEVELS:
        sx = np.arange(w, dtype=np.float32) + 0.5
        sy = np.arange(h, dtype=np.float32) + 0.5
        gy, gx = np.meshgrid(sy, sx, indexing="ij")
        pts.append(np.stack([gx, gy], -1).reshape(-1, 2))
        strs.append(np.full((h * w, 1), stride, dtype=np.float32))
    return np.concatenate(pts), np.concatenate(strs)


def _cxcywh_to_xyxy(b):
    cx, cy, w, h = b[..., 0], b[..., 1], b[..., 2], b[..., 3]
    return np.stack([cx - w / 2, cy - h / 2, cx + w / 2, cy + h / 2], axis=-1)


def _giou_elementwise(a, b):
    lt = np.maximum(a[..., :2], b[..., :2])
    rb = np.minimum(a[..., 2:], b[..., 2:])
    wh = np.maximum(rb - lt, 0.0)
    inter = wh[..., 0] * wh[..., 1]
    area_a = (a[..., 2] - a[..., 0]) * (a[..., 3] - a[..., 1])
    area_b = (b[..., 2] - b[..., 0]) * (b[..., 3] - b[..., 1])
    union = area_a + area_b - inter
    iou = inter / union
    lt_c = np.minimum(a[..., :2], b[..., :2])
    rb_c = np.maximum(a[..., 2:], b[..., 2:])
    wh_c = np.maximum(rb_c - lt_c, 0.0)
    area_c = wh_c[..., 0] * wh_c[..., 1]
    return iou - (area_c - union) / area_c


def _focal_f32(x, t):
    """Reference focal loss term, elementwise, f64 math on f32 inputs."""
    x = x.astype(np.float64)
    bce = np.maximum(x, 0.0) - x * t + np.log1p(np.exp(-np.abs(x)))
    pt = np.exp(-bce)
    return 0.25 * (1.0 - pt) ** 2 * bce


# ------------------------------------------------------------------- kernel
def kernel(pred_boxes, pred_scores, targets_bbox, targets_cls):
    pred_boxes = np.asarray(pred_boxes, dtype=np.float32)
    pred_scores = np.ascontiguousarray(np.asarray(pred_scores, dtype=np.float32))
    targets_bbox = np.asarray(targets_bbox, dtype=np.float32)
    targets_cls = np.asarray(targets_cls)

    # ---- device: sum of focal(x, t=0) over all of pred_scores ----
    res = _run_device(_make_in_maps(pred_scores))
    focal0_total = float(
        sum(float(r["acc_out"][0, 0]) for r in res.results)
    )

    # ---- host: top-k anchor matching (depends only on targets_bbox) ----
    anchors, stride_t = _make_anchors()                    # [A,2], [A,1] f32
    centers = anchors * stride_t                           # [A,2] f32
    diff = centers[None, :, :] - targets_bbox[:, None, :2]  # [B,A,2] f32
    dist = np.sqrt(diff[..., 0] * diff[..., 0] + diff[..., 1] * diff[..., 1])
    topk_idx = np.argpartition(dist, _TOPK, axis=1)[:, :_TOPK]  # [B,K]

    bi = np.arange(_B)[:, None]
    # ---- host: GIoU box loss on the K matched anchors per batch row ----
    pb_g = pred_boxes.transpose(0, 2, 1)[bi, topk_idx]      # [B,K,4] f32
    anc_g = anchors[topk_idx]                               # [B,K,2]
    str_g = stride_t[topk_idx]                              # [B,K,1]
    pred_cxcy = (anc_g + pb_g[..., :2]) * str_g
    pred_wh = np.exp(np.minimum(pb_g[..., 2:], 10.0)) * str_g
    decoded = np.concatenate([pred_cxcy, pred_wh], axis=-1).astype(np.float32)
    pred_xyxy = _cxcywh_to_xyxy(decoded)
    gt_xyxy = _cxcywh_to_xyxy(targets_bbox)[:, None, :]
    giou = _giou_elementwise(
        pred_xyxy.astype(np.float64),
        np.broadcast_to(gt_xyxy, pred_xyxy.shape).astype(np.float64),
    )
    loss_box = (1.0 - giou).mean(axis=1).mean()

    # ---- host: focal correction at the K matched (anchor, class) slots ----
    cls_idx = targets_cls.astype(np.int64)[:, None]         # [B,1]
    xg = pred_scores[bi, cls_idx, topk_idx]                 # [B,K] f32
    corr = (_focal_f32(xg, 1.0) - _focal_f32(xg, 0.0)).sum()

    loss_cls = (focal0_total + corr) / _B
    total = 5.0 * loss_box + 1.0 * loss_cls
    return (
        np.float32(total),
        np.float32(loss_box),
        np.float32(loss_cls),
    )
